# revision 24
# baseline (speedup 1.0000x reference)
"""BEV feature extractor (bilinear sampling) as a Trainium2 Bass kernel.

Full-I/O contract: kernel(bev_features=(4,180,180,256) f32,
batch_centers=(4,10240,2) f32) -> (4,2048,1280) f32.

Sharding: data-parallel. Batch b maps to cores (2b, 2b+1); each core
processes 5120 of the batch's 10240 sample points against that batch's
full BEV map. No cross-core communication.

HW profiling showed the gather is bound by per-descriptor cost (SWDGE
descriptor generation + HBM random-read overhead), not just bytes, so
the host pre-expands the image into 2x2 corner PATCHES:
P[y, x] = [im[y,x], im[y,xc], im[yc,x], im[yc,xc]] (xc/yc = clamped +1),
2KB contiguous fp16 per patch. One gather descriptor then fetches all
four bilinear corners of a point (5120 descriptors/core instead of
10240/core row pairs), with the reference's clamp semantics baked into
the patch construction. fp16 (gate is 2e-2; fp16 adds ~6e-4) halves
bytes vs f32: 5120 x 2KB = 10.5MB read + 2.6MB fp16 store per core.

Device kernel per core:
  - preamble: load centers point-major ([128, J] layout), compute the
    four bilinear weights (f32) and int16 patch indices y0*180+x0. The
    gather ucode wants indices wrapped [16, i//16] replicated across the
    8 Q7 groups; that partition shuffle bounces the [128, J] int16 index
    tile through a DRAM scratch and re-loads with a strided AP.
  - main loop (10 x 512 points): gpsimd.dma_gather pulls each point's
    patch (4 x 256ch fp16) into the point's SBUF partition; 2 custom-DVE
    MAC2 ops per 128-point tile apply the bilinear weights (fp16 in,
    fp16 out, f32 per-partition scalar weights), one wide fp16
    tensor_tensor add combines, HWDGE stores fp16 (stores alternate
    between the SP and ACT HWDGE rings).

The NUM_POINT interleave (out[b, r, p*256:(p+1)*256] = raw[b, p*2048+r])
is pure data movement, applied host-side while unsharding.
"""

import sys

for _p in ("/opt/trn_rl_repo", "/root/.axon_site/_ro/trn_rl_repo"):
    if _p not in sys.path:
        sys.path.append(_p)

import numpy as np

B = 4
H = W = 180
C = 256
N = 10240
NUM_POINT = 5
SEC = N // NUM_POINT  # 2048
NCORES = 8
PTS = N // 2          # points per core: 5120
NJ = PTS // 128       # 40 point-tiles per core
NPATCH = H * W        # 32400 patch indices (fits int16)

INV_VOX = 0.075
CLMAX = float(W - 1)  # 179.0
RND_MAGIC = 2.0**23

_CACHE = {}


def _register_mac2():
    """Custom fused DVE op: out = in0*s0 + in1*s1 (s0/s1 per-partition scalars).
    Replaces two scalar_tensor_tensor MACs with one 1x-mode instruction."""
    import numpy as np
    from concourse.dve_spec import Spec, Src0, Src1, C0, C1, lower
    from concourse.dve_ops import (
        DveOp, OPS, _SUB_OPCODE_FOR_NAME, _CUSTOM_DVE_ROW_BASE,
        CUSTOM_DVE_SPECS, get_dve_sub_opcode,
    )
    from concourse.dve_uop import DveOpSpec
    from concourse.dve_table_gen import dve_ver_for

    name = "MAC2_BILIN_ANT"
    for op in OPS:
        if op.name == name:
            return op
    spec = Spec(
        body=Src0 * C0 + Src1 * C1,
        reference=lambda in0, in1, s0, s1, imm2: (in0 * s0 + in1 * s1).astype(
            np.float32
        ),
    )
    op = DveOp(name, spec, subdim=False, uops_sha={})
    OPS.append(op)
    _SUB_OPCODE_FOR_NAME[name] = _CUSTOM_DVE_ROW_BASE + len(OPS) - 1
    CUSTOM_DVE_SPECS[name] = spec
    for trn in ("TRN2",):
        ver = dve_ver_for(trn)
        uops = lower(spec, ver=ver)
        op.uops_sha[ver] = DveOpSpec(
            name=name, opcode=get_dve_sub_opcode(name), uops=uops, rd1_en=True
        ).sha(ver)
    return op


def _build_program(loop_repeat=1, compute_mode="mac2f16", kj=4, gbufs=6, abufs=6,
                   single_packet=False, chain=1, nqueues=1, io_dtype="f16",
                   indirect=False):
    import concourse.tile as tile
    from concourse import bacc, mybir
    from concourse.bass import AP, IndirectOffsetOnAxis

    f32 = mybir.dt.float32
    i16 = mybir.dt.int16
    fio = f32 if io_dtype == "f32" else mybir.dt.float16
    facc = fio if compute_mode == "mac2f16" else f32
    Op = mybir.AluOpType
    recip = float(np.float32(1.0) / np.float32(INV_VOX))
    mac2 = _register_mac2()

    nc = bacc.Bacc(
        "TRN2",
        target_bir_lowering=False,
        debug=False,
        enable_asserts=False,
        num_devices=NCORES,
        num_swdge_queues=nqueues,
    )
    img = nc.dram_tensor("img", (NPATCH, 4 * C), fio, kind="ExternalInput").ap()
    ctr = nc.dram_tensor("ctr", (PTS, 2), f32, kind="ExternalInput").ap()
    out = nc.dram_tensor("out", (PTS, C), fio, kind="ExternalOutput").ap()
    # scratch for the natural->wrapped partition shuffle of gather indices
    scr = nc.dram_tensor("scr", (128, NJ), i16, kind="Internal").ap()

    nk = NJ // kj
    ni = 128 * kj  # gather indices per call (one per point)
    with tile.TileContext(nc) as tc:
        with (
            tc.tile_pool(name="const", bufs=1 if chain == 1 else 2) as cpool,
            tc.tile_pool(name="gather", bufs=gbufs) as gpool,
            tc.tile_pool(name="accum", bufs=abufs) as apool,
        ):
          # chain > 1 re-emits the whole program body (timing-only mode):
          # tiles rotate through their pools with dependency tracking, so the
          # repeats serialize on-device and (T(chain_hi)-T(chain_lo))/Δchain
          # isolates one full execution from dispatch/tunnel overhead.
          for _chain in range(chain):
            # ============ preamble: weights + gather indices ============
            # ctrB[p, 2J+c] = centers[128J + p, c]
            ctrB = cpool.tile([128, 2 * NJ], f32)
            nc.sync.dma_start(
                ctrB[:].rearrange("p (J c) -> p J c", c=2),
                ctr.rearrange("(J p) c -> p J c", p=128),
            )
            coord = cpool.tile([128, 2 * NJ], f32)
            nc.vector.tensor_scalar(coord[:], ctrB[:], 54.0, recip, Op.add, Op.mult)
            nc.vector.tensor_scalar(coord[:], coord[:], 0.125, None, Op.mult)
            # F0 = clip(floor(coord), 0, 179) via the 2^23 round trick
            rnd = cpool.tile([128, 2 * NJ], f32)
            nc.vector.tensor_scalar(rnd[:], coord[:], RND_MAGIC, None, Op.add)
            nc.vector.tensor_scalar(rnd[:], rnd[:], RND_MAGIC, None, Op.subtract)
            gtf = cpool.tile([128, 2 * NJ], f32)
            nc.vector.tensor_tensor(gtf[:], rnd[:], coord[:], Op.is_gt)
            flo = cpool.tile([128, 2 * NJ], f32)
            nc.vector.tensor_tensor(flo[:], rnd[:], gtf[:], Op.subtract)
            F0 = cpool.tile([128, 2 * NJ], f32)
            nc.vector.tensor_scalar(F0[:], flo[:], 0.0, CLMAX, Op.max, Op.min)

            # patch indices first -- they gate the first gather; the weights
            # (below) are only needed once gathered data lands.
            # idx16n[p, J] = y0*180 + x0 in the natural layout.
            idxf = cpool.tile([128, NJ], f32)
            nc.vector.scalar_tensor_tensor(
                idxf[:], F0[:, 1::2], float(W), F0[:, 0::2], Op.mult, Op.add
            )
            if indirect:
                idxi32 = cpool.tile([128, NJ], mybir.dt.int32)
                nc.vector.tensor_copy(idxi32[:], idxf[:])
            idx16n = cpool.tile([128, NJ], i16)
            nc.vector.tensor_copy(idx16n[:], idxf[:])

            # natural -> wrapped shuffle through DRAM scratch.
            # dma_gather linear index i (of call k) lands on out partition
            # i%128, slot i//128, and is read from idxs[i%16, i//16]
            # (replicated across the 8 Q7 partition groups). With
            # i = 128*j + 16*p1 + q for point 128(kj*k+j)+16p1+q, the
            # wrapped tile needs idx16w[16g+q, 8*(kj*k+j)+p1] =
            # idx16n[16p1+q, kj*k+j] -- for fixed (q,p1) contiguous along
            # (k,j) on the scratch side (80B descriptor runs); one load per
            # replica group g (AP balancer caps at 3 dims), alternating
            # between the SP and ACT HWDGE rings.
            nc.sync.dma_start(scr, idx16n[:])
            idx16w = cpool.tile([128, 8 * NJ], i16)
            src = AP(scr.tensor, 0, [[NJ, 16], [1, NJ], [16 * NJ, 8]])
            for g in range(8):
                eng = nc.sync if g % 2 == 0 else nc.scalar
                eng.dma_start(
                    idx16w[16 * g : 16 * (g + 1), :].rearrange(
                        "q (a p1) -> q a p1", p1=8
                    ),
                    src,
                )

            F1 = cpool.tile([128, 2 * NJ], f32)
            nc.vector.tensor_scalar(F1[:], F0[:], 1.0, CLMAX, Op.add, Op.min)
            fxy = cpool.tile([128, 2 * NJ], f32)
            nc.vector.tensor_tensor(fxy[:], F1[:], F0[:], Op.subtract)
            wBt = cpool.tile([128, 2 * NJ], f32)
            nc.vector.tensor_tensor(wBt[:], coord[:], F0[:], Op.subtract)
            nc.vector.tensor_tensor(wBt[:], wBt[:], fxy[:], Op.mult)
            wAt = cpool.tile([128, 2 * NJ], f32)
            nc.vector.tensor_tensor(wAt[:], fxy[:], wBt[:], Op.subtract)
            w00 = cpool.tile([128, NJ], f32)
            w01 = cpool.tile([128, NJ], f32)
            w10 = cpool.tile([128, NJ], f32)
            w11 = cpool.tile([128, NJ], f32)
            nc.vector.tensor_tensor(w00[:], wAt[:, 0::2], wAt[:, 1::2], Op.mult)
            nc.vector.tensor_tensor(w01[:], wBt[:, 0::2], wAt[:, 1::2], Op.mult)
            nc.vector.tensor_tensor(w10[:], wAt[:, 0::2], wBt[:, 1::2], Op.mult)
            nc.vector.tensor_tensor(w11[:], wBt[:, 0::2], wBt[:, 1::2], Op.mult)

            # ============ main loop ============
            # loop_repeat > 1 is a timing-only mode (see chain above).
            in_ap = AP(img.tensor, 0, [[4 * C, NPATCH], [1, 4 * C]])
            for k in [kk for _ in range(loop_repeat) for kk in range(nk)]:
                gt = gpool.tile([128, kj * 4 * C], fio)
                if indirect:
                    # Negative result, kept for the record: indirect_dma_start
                    # with per-(p,j) offsets is correct in CoreSim and ~10%
                    # faster in the cost model, but wedges the mesh on real HW
                    # ("mesh desynced"). Do not enable.
                    nc.gpsimd.indirect_dma_start(
                        out=gt[:].rearrange("p (g e) -> p g e", e=4 * C),
                        out_offset=None,
                        in_=in_ap,
                        in_offset=IndirectOffsetOnAxis(
                            ap=idxi32[:, kj * k : kj * (k + 1)], axis=0
                        ),
                    )
                else:
                    nc.gpsimd.dma_gather(
                        out_ap=gt[:].rearrange("p (g e) -> p g e", e=4 * C),
                        in_ap=in_ap,
                        idxs_ap=idx16w[:, (ni // 16) * k : (ni // 16) * (k + 1)],
                        num_idxs=ni,
                        num_idxs_reg=ni,
                        elem_size=4 * C,
                        single_packet=single_packet,
                        queue_num=k % nqueues,
                    )
                if compute_mode == "gatherstore":
                    nc.sync.dma_start(
                        out.rearrange("(k j p) c -> k p j c", p=128, j=kj)[k],
                        gt[:, 0 : kj * C].rearrange("p (j c) -> p j c", c=C),
                    )
                    continue
                acc = apool.tile([128, kj * C], facc)
                accB = apool.tile([128, kj * C], facc)
                for j in range(kj):
                    J = kj * k + j
                    v = gt[:, j * 4 * C : (j + 1) * 4 * C]
                    a = acc[:, j * C : (j + 1) * C]
                    b = accB[:, j * C : (j + 1) * C]
                    nc.vector._custom_dve(
                        mac2, out=a, in0=v[:, 0:C], in1=v[:, C : 2 * C],
                        s0=w00[:, J : J + 1], s1=w01[:, J : J + 1],
                    )
                    nc.vector._custom_dve(
                        mac2, out=b, in0=v[:, 2 * C : 3 * C], in1=v[:, 3 * C : 4 * C],
                        s0=w10[:, J : J + 1], s1=w11[:, J : J + 1],
                    )
                # one wide combine per gather call
                if facc is fio and fio is f32:
                    nc.vector.tensor_tensor(acc[:], acc[:], accB[:], Op.add)
                    o = acc
                else:
                    o = apool.tile([128, kj * C], fio)
                    nc.vector.tensor_tensor(o[:], acc[:], accB[:], Op.add)
                # out rows: row = 128*kj*k + 128j + p
                dst = out.rearrange("(k j p) c -> k p j c", p=128, j=kj)[k]
                seng = nc.sync if k % 2 == 0 else nc.scalar
                seng.dma_start(dst, o[:].rearrange("p (j c) -> p j c", c=C))

    nc.compile()
    return nc


IO_DTYPE = "f16"


def _get_program():
    if "nc" not in _CACHE:
        _CACHE["nc"] = _build_program(io_dtype=IO_DTYPE)
    return _CACHE["nc"]


def _expand_patches(im, np_io):
    """im: (H, W, C) f32 -> (H*W, 4*C) np_io patch tensor with clamped
    +1 neighbors (matches the reference's clamped corner indexing)."""
    imc = im.astype(np_io)
    xs = np.minimum(np.arange(W) + 1, W - 1)
    ys = np.minimum(np.arange(H) + 1, H - 1)
    p = np.empty((H, W, 4, C), dtype=np_io)
    p[:, :, 0, :] = imc
    p[:, :, 1, :] = imc[:, xs]
    p[:, :, 2, :] = imc[ys, :]
    p[:, :, 3, :] = imc[ys][:, xs]
    return p.reshape(NPATCH, 4 * C)


def _make_in_maps(bev_features, batch_centers, io_dtype=IO_DTYPE):
    np_io = np.float32 if io_dtype == "f32" else np.float16
    bev = np.ascontiguousarray(np.asarray(bev_features, dtype=np.float32))
    cen = np.ascontiguousarray(np.asarray(batch_centers, dtype=np.float32))
    assert bev.shape == (B, H, W, C) and cen.shape == (B, N, 2)
    patches = [_expand_patches(bev[b], np_io) for b in range(B)]
    in_maps = []
    for core in range(NCORES):
        b, h = core // 2, core % 2
        in_maps.append(
            {
                "img": patches[b],
                "ctr": cen[b, h * PTS : (h + 1) * PTS, :],
            }
        )
    return in_maps


def _unshard(results):
    # results[core]["out"]: (5120, 256) in raw point order
    final = np.empty((B, SEC, NUM_POINT * C), dtype=np.float32)
    for b in range(B):
        raw = np.concatenate(
            [np.asarray(results[2 * b]["out"], dtype=np.float32),
             np.asarray(results[2 * b + 1]["out"], dtype=np.float32)], axis=0)
        # out[b, r, p*C:(p+1)*C] = raw[p*SEC + r]
        final[b] = (
            raw.reshape(NUM_POINT, SEC, C).transpose(1, 0, 2).reshape(SEC, NUM_POINT * C)
        )
    return final


def run_on_hw(bev_features, batch_centers, trace=False):
    """Run the SPMD kernel on the 8 NeuronCores; returns (output, BassKernelResults)."""
    from concourse.bass_utils import run_bass_kernel_spmd

    nc = _get_program()
    in_maps = _make_in_maps(bev_features, batch_centers)
    res = run_bass_kernel_spmd(nc, in_maps, core_ids=list(range(NCORES)), trace=trace)
    return _unshard(res.results), res


def kernel(bev_features, batch_centers):
    out, _ = run_on_hw(bev_features, batch_centers, trace=False)
    return out


# revision 26
# speedup vs baseline: 1.1402x; 1.1402x over previous
"""BEV feature extractor (bilinear sampling) as a Trainium2 Bass kernel.

Full-I/O contract: kernel(bev_features=(4,180,180,256) f32,
batch_centers=(4,10240,2) f32) -> (4,2048,1280) f32.

Sharding: data-parallel. Batch b maps to cores (2b, 2b+1); each core
processes 5120 of the batch's 10240 sample points against that batch's
full BEV map. No cross-core communication.

HW profiling showed the gather is bound by per-descriptor cost (SWDGE
descriptor generation + HBM random-read overhead), not just bytes, so
the host pre-expands the image into 2x2 corner PATCHES:
P[y, x] = [im[y,x], im[y,xc], im[yc,x], im[yc,xc]] (xc/yc = clamped +1),
2KB contiguous fp16 per patch. One gather descriptor then fetches all
four bilinear corners of a point (5120 descriptors/core instead of
10240/core row pairs), with the reference's clamp semantics baked into
the patch construction. fp16 (gate is 2e-2; fp16 adds ~6e-4) halves
bytes vs f32: 5120 x 2KB = 10.5MB read + 2.6MB fp16 store per core.

Device kernel per core:
  - preamble: load centers point-major ([128, J] layout), compute the
    four bilinear weights (f32) and int16 patch indices y0*180+x0. The
    gather ucode wants indices wrapped [16, i//16] replicated across the
    8 Q7 groups; that partition shuffle bounces the [128, J] int16 index
    tile through a DRAM scratch and re-loads with a strided AP.
  - main loop (10 x 512 points): gpsimd.dma_gather pulls each point's
    patch (4 x 256ch fp16) into the point's SBUF partition; 2 custom-DVE
    MAC2 ops per 128-point tile apply the bilinear weights (fp16 in,
    fp16 out, f32 per-partition scalar weights), one wide fp16
    tensor_tensor add combines, HWDGE stores fp16 (stores alternate
    between the SP and ACT HWDGE rings).

The NUM_POINT interleave (out[b, r, p*256:(p+1)*256] = raw[b, p*2048+r])
is pure data movement, applied host-side while unsharding.
"""

import sys

for _p in ("/opt/trn_rl_repo", "/root/.axon_site/_ro/trn_rl_repo"):
    if _p not in sys.path:
        sys.path.append(_p)

import numpy as np

B = 4
H = W = 180
C = 256
N = 10240
NUM_POINT = 5
SEC = N // NUM_POINT  # 2048
NCORES = 8
PTS = N // 2          # points per core: 5120
NJ = PTS // 128       # 40 point-tiles per core
NPATCH = H * W        # 32400 patch indices (fits int16)

INV_VOX = 0.075
CLMAX = float(W - 1)  # 179.0
RND_MAGIC = 2.0**23

_CACHE = {}


def _register_mac2():
    """Custom fused DVE op: out = in0*s0 + in1*s1 (s0/s1 per-partition scalars).
    Replaces two scalar_tensor_tensor MACs with one 1x-mode instruction."""
    import numpy as np
    from concourse.dve_spec import Spec, Src0, Src1, C0, C1, lower
    from concourse.dve_ops import (
        DveOp, OPS, _SUB_OPCODE_FOR_NAME, _CUSTOM_DVE_ROW_BASE,
        CUSTOM_DVE_SPECS, get_dve_sub_opcode,
    )
    from concourse.dve_uop import DveOpSpec
    from concourse.dve_table_gen import dve_ver_for

    name = "MAC2_BILIN_ANT"
    for op in OPS:
        if op.name == name:
            return op
    spec = Spec(
        body=Src0 * C0 + Src1 * C1,
        reference=lambda in0, in1, s0, s1, imm2: (in0 * s0 + in1 * s1).astype(
            np.float32
        ),
    )
    op = DveOp(name, spec, subdim=False, uops_sha={})
    OPS.append(op)
    _SUB_OPCODE_FOR_NAME[name] = _CUSTOM_DVE_ROW_BASE + len(OPS) - 1
    CUSTOM_DVE_SPECS[name] = spec
    for trn in ("TRN2",):
        ver = dve_ver_for(trn)
        uops = lower(spec, ver=ver)
        op.uops_sha[ver] = DveOpSpec(
            name=name, opcode=get_dve_sub_opcode(name), uops=uops, rd1_en=True
        ).sha(ver)
    return op


def _build_program(loop_repeat=1, compute_mode="mac2f16", kj=4, gbufs=6, abufs=6,
                   single_packet=False, chain=1, nqueues=1, io_dtype="f16",
                   indirect=False):
    import concourse.tile as tile
    from concourse import bacc, mybir
    from concourse.bass import AP, IndirectOffsetOnAxis

    f32 = mybir.dt.float32
    i16 = mybir.dt.int16
    fio = f32 if io_dtype == "f32" else mybir.dt.float16
    facc = fio if compute_mode == "mac2f16" else f32
    Op = mybir.AluOpType
    recip = float(np.float32(1.0) / np.float32(INV_VOX))
    mac2 = _register_mac2()

    nc = bacc.Bacc(
        "TRN2",
        target_bir_lowering=False,
        debug=False,
        enable_asserts=False,
        num_devices=NCORES,
        num_swdge_queues=nqueues,
    )
    img = nc.dram_tensor("img", (NPATCH, 4 * C), fio, kind="ExternalInput").ap()
    ctr = nc.dram_tensor("ctr", (PTS, 2), f32, kind="ExternalInput").ap()
    out = nc.dram_tensor("out", (PTS, C), fio, kind="ExternalOutput").ap()
    # scratch for the natural->wrapped partition shuffle of gather indices
    scr = nc.dram_tensor("scr", (128, NJ), i16, kind="Internal").ap()

    nk = NJ // kj
    ni = 128 * kj  # gather indices per call (one per point)
    with tile.TileContext(nc) as tc:
        with (
            tc.tile_pool(name="const", bufs=1 if chain == 1 else 2) as cpool,
            tc.tile_pool(name="gather", bufs=gbufs) as gpool,
            tc.tile_pool(name="accum", bufs=abufs) as apool,
        ):
          # chain > 1 re-emits the whole program body (timing-only mode):
          # tiles rotate through their pools with dependency tracking, so the
          # repeats serialize on-device and (T(chain_hi)-T(chain_lo))/Δchain
          # isolates one full execution from dispatch/tunnel overhead.
          for _chain in range(chain):
            # ============ preamble: weights + gather indices ============
            # ctrB[p, 2J+c] = centers[128J + p, c]
            ctrB = cpool.tile([128, 2 * NJ], f32)
            nc.sync.dma_start(
                ctrB[:].rearrange("p (J c) -> p J c", c=2),
                ctr.rearrange("(J p) c -> p J c", p=128),
            )
            coord = cpool.tile([128, 2 * NJ], f32)
            nc.vector.tensor_scalar(coord[:], ctrB[:], 54.0, recip, Op.add, Op.mult)
            nc.vector.tensor_scalar(coord[:], coord[:], 0.125, None, Op.mult)
            # F0 = clip(floor(coord), 0, 179) via the 2^23 round trick
            rnd = cpool.tile([128, 2 * NJ], f32)
            nc.vector.tensor_scalar(rnd[:], coord[:], RND_MAGIC, None, Op.add)
            nc.vector.tensor_scalar(rnd[:], rnd[:], RND_MAGIC, None, Op.subtract)
            gtf = cpool.tile([128, 2 * NJ], f32)
            nc.vector.tensor_tensor(gtf[:], rnd[:], coord[:], Op.is_gt)
            flo = cpool.tile([128, 2 * NJ], f32)
            nc.vector.tensor_tensor(flo[:], rnd[:], gtf[:], Op.subtract)
            F0 = cpool.tile([128, 2 * NJ], f32)
            nc.vector.tensor_scalar(F0[:], flo[:], 0.0, CLMAX, Op.max, Op.min)

            # patch indices first -- they gate the first gather; the weights
            # (below) are only needed once gathered data lands.
            # idx16n[p, J] = y0*180 + x0 in the natural layout.
            idxf = cpool.tile([128, NJ], f32)
            nc.vector.scalar_tensor_tensor(
                idxf[:], F0[:, 1::2], float(W), F0[:, 0::2], Op.mult, Op.add
            )
            if indirect:
                idxi32 = cpool.tile([128, NJ], mybir.dt.int32)
                nc.vector.tensor_copy(idxi32[:], idxf[:])
            idx16n = cpool.tile([128, NJ], i16)
            nc.vector.tensor_copy(idx16n[:], idxf[:])

            # natural -> wrapped shuffle through DRAM scratch.
            # dma_gather linear index i (of call k) lands on out partition
            # i%128, slot i//128, and is read from idxs[i%16, i//16]
            # (replicated across the 8 Q7 partition groups). With
            # i = 128*j + 16*p1 + q for point 128(kj*k+j)+16p1+q, the
            # wrapped tile needs idx16w[16g+q, 8*(kj*k+j)+p1] =
            # idx16n[16p1+q, kj*k+j] -- for fixed (q,p1) contiguous along
            # (k,j) on the scratch side (80B descriptor runs); one load per
            # replica group g (AP balancer caps at 3 dims), alternating
            # between the SP and ACT HWDGE rings.
            nc.sync.dma_start(scr, idx16n[:])
            idx16w = cpool.tile([128, 8 * NJ], i16)
            src = AP(scr.tensor, 0, [[NJ, 16], [1, NJ], [16 * NJ, 8]])
            for g in range(8):
                eng = nc.sync if g % 2 == 0 else nc.scalar
                eng.dma_start(
                    idx16w[16 * g : 16 * (g + 1), :].rearrange(
                        "q (a p1) -> q a p1", p1=8
                    ),
                    src,
                )

            F1 = cpool.tile([128, 2 * NJ], f32)
            nc.vector.tensor_scalar(F1[:], F0[:], 1.0, CLMAX, Op.add, Op.min)
            fxy = cpool.tile([128, 2 * NJ], f32)
            nc.vector.tensor_tensor(fxy[:], F1[:], F0[:], Op.subtract)
            wBt = cpool.tile([128, 2 * NJ], f32)
            nc.vector.tensor_tensor(wBt[:], coord[:], F0[:], Op.subtract)
            nc.vector.tensor_tensor(wBt[:], wBt[:], fxy[:], Op.mult)
            wAt = cpool.tile([128, 2 * NJ], f32)
            nc.vector.tensor_tensor(wAt[:], fxy[:], wBt[:], Op.subtract)
            w00 = cpool.tile([128, NJ], f32)
            w01 = cpool.tile([128, NJ], f32)
            w10 = cpool.tile([128, NJ], f32)
            w11 = cpool.tile([128, NJ], f32)
            nc.vector.tensor_tensor(w00[:], wAt[:, 0::2], wAt[:, 1::2], Op.mult)
            nc.vector.tensor_tensor(w01[:], wBt[:, 0::2], wAt[:, 1::2], Op.mult)
            nc.vector.tensor_tensor(w10[:], wAt[:, 0::2], wBt[:, 1::2], Op.mult)
            nc.vector.tensor_tensor(w11[:], wBt[:, 0::2], wBt[:, 1::2], Op.mult)

            # ============ main loop ============
            # loop_repeat > 1 is a timing-only mode (see chain above).
            in_ap = AP(img.tensor, 0, [[4 * C, NPATCH], [1, 4 * C]])
            for k in [kk for _ in range(loop_repeat) for kk in range(nk)]:
                gt = gpool.tile([128, kj * 4 * C], fio)
                if indirect:
                    # Negative result, kept for the record: indirect_dma_start
                    # with per-(p,j) offsets is correct in CoreSim and ~10%
                    # faster in the cost model, but wedges the mesh on real HW
                    # ("mesh desynced"). Do not enable.
                    nc.gpsimd.indirect_dma_start(
                        out=gt[:].rearrange("p (g e) -> p g e", e=4 * C),
                        out_offset=None,
                        in_=in_ap,
                        in_offset=IndirectOffsetOnAxis(
                            ap=idxi32[:, kj * k : kj * (k + 1)], axis=0
                        ),
                    )
                else:
                    nc.gpsimd.dma_gather(
                        out_ap=gt[:].rearrange("p (g e) -> p g e", e=4 * C),
                        in_ap=in_ap,
                        idxs_ap=idx16w[:, (ni // 16) * k : (ni // 16) * (k + 1)],
                        num_idxs=ni,
                        num_idxs_reg=ni,
                        elem_size=4 * C,
                        single_packet=single_packet,
                        queue_num=k % nqueues,
                    )
                if compute_mode == "gatherstore":
                    nc.sync.dma_start(
                        out.rearrange("(k j p) c -> k p j c", p=128, j=kj)[k],
                        gt[:, 0 : kj * C].rearrange("p (j c) -> p j c", c=C),
                    )
                    continue
                acc = apool.tile([128, kj * C], facc)
                accB = apool.tile([128, kj * C], facc)
                for j in range(kj):
                    J = kj * k + j
                    v = gt[:, j * 4 * C : (j + 1) * 4 * C]
                    a = acc[:, j * C : (j + 1) * C]
                    b = accB[:, j * C : (j + 1) * C]
                    nc.vector._custom_dve(
                        mac2, out=a, in0=v[:, 0:C], in1=v[:, C : 2 * C],
                        s0=w00[:, J : J + 1], s1=w01[:, J : J + 1],
                    )
                    nc.vector._custom_dve(
                        mac2, out=b, in0=v[:, 2 * C : 3 * C], in1=v[:, 3 * C : 4 * C],
                        s0=w10[:, J : J + 1], s1=w11[:, J : J + 1],
                    )
                # one wide combine per gather call
                if facc is fio and fio is f32:
                    nc.vector.tensor_tensor(acc[:], acc[:], accB[:], Op.add)
                    o = acc
                else:
                    o = apool.tile([128, kj * C], fio)
                    nc.vector.tensor_tensor(o[:], acc[:], accB[:], Op.add)
                # out rows: row = 128*kj*k + 128j + p
                dst = out.rearrange("(k j p) c -> k p j c", p=128, j=kj)[k]
                seng = nc.sync if k % 2 == 0 else nc.scalar
                seng.dma_start(dst, o[:].rearrange("p (j c) -> p j c", c=C))

    nc.compile()
    return nc


IO_DTYPE = "f16"


def _get_program():
    if "nc" not in _CACHE:
        _CACHE["nc"] = _build_program(io_dtype=IO_DTYPE)
    return _CACHE["nc"]


def _expand_patches(im, np_io):
    """im: (H, W, C) f32 -> (H*W, 4*C) np_io patch tensor with clamped
    +1 neighbors (matches the reference's clamped corner indexing)."""
    imc = im.astype(np_io)
    xs = np.minimum(np.arange(W) + 1, W - 1)
    ys = np.minimum(np.arange(H) + 1, H - 1)
    p = np.empty((H, W, 4, C), dtype=np_io)
    p[:, :, 0, :] = imc
    p[:, :, 1, :] = imc[:, xs]
    p[:, :, 2, :] = imc[ys, :]
    p[:, :, 3, :] = imc[ys][:, xs]
    return p.reshape(NPATCH, 4 * C)


def _make_in_maps(bev_features, batch_centers, io_dtype=IO_DTYPE):
    np_io = np.float32 if io_dtype == "f32" else np.float16
    bev = np.ascontiguousarray(np.asarray(bev_features, dtype=np.float32))
    cen = np.ascontiguousarray(np.asarray(batch_centers, dtype=np.float32))
    assert bev.shape == (B, H, W, C) and cen.shape == (B, N, 2)
    patches = [_expand_patches(bev[b], np_io) for b in range(B)]
    in_maps = []
    for core in range(NCORES):
        b, h = core // 2, core % 2
        in_maps.append(
            {
                "img": patches[b],
                "ctr": cen[b, h * PTS : (h + 1) * PTS, :],
            }
        )
    return in_maps


def _unshard(results):
    # results[core]["out"]: (5120, 256) in raw point order
    final = np.empty((B, SEC, NUM_POINT * C), dtype=np.float32)
    for b in range(B):
        raw = np.concatenate(
            [np.asarray(results[2 * b]["out"], dtype=np.float32),
             np.asarray(results[2 * b + 1]["out"], dtype=np.float32)], axis=0)
        # out[b, r, p*C:(p+1)*C] = raw[p*SEC + r]
        final[b] = (
            raw.reshape(NUM_POINT, SEC, C).transpose(1, 0, 2).reshape(SEC, NUM_POINT * C)
        )
    return final


def run_on_hw(bev_features, batch_centers, trace=False):
    """Run the SPMD kernel on the 8 NeuronCores; returns (output, BassKernelResults)."""
    from concourse.bass_utils import run_bass_kernel_spmd

    nc = _get_program()
    in_maps = _make_in_maps(bev_features, batch_centers)
    res = run_bass_kernel_spmd(nc, in_maps, core_ids=list(range(NCORES)), trace=trace)
    return _unshard(res.results), res


def kernel(bev_features, batch_centers):
    out, _ = run_on_hw(bev_features, batch_centers, trace=False)
    return out
